# revision 9
# baseline (speedup 1.0000x reference)
"""Trainium2 Bass kernel for a dense transformer block (B=4, T=1024, C=1024, H=16).

Sharding: 8 cores = 4 batches x 2 tensor-parallel groups.
  Phase A (attention): core (b, g) computes LN1 + its 8 heads of attention +
    the partial output projection -> projT partial [C, T].
    Host combines: x2 = x + projT_even.T + projT_odd.T + bp.
  Phase B (FFN): core (b, g) computes LN2 + its half (2048) of the FFN hidden
    dim -> ffpT partial [C, T].
    Host combines: out = x2 + ffpT_even.T + ffpT_odd.T + b2.

LayerNorm in this model normalizes over the SEQUENCE axis (dim=1 of [B,T,C]),
so all on-chip tensors live in [C, T] ("transposed") layout where that
reduction is a free-axis reduction.
"""
import sys
import os

sys.path.insert(0, "/opt/trn_rl_repo")

import numpy as np
import ml_dtypes
from contextlib import ExitStack

import concourse.bacc as bacc
import concourse.mybir as mybir
import concourse.tile as tile

bf16 = mybir.dt.bfloat16
f32 = mybir.dt.float32

B, T, C, H = 4, 1024, 1024, 16
HD = 64                    # head dim
NHG = 8                    # heads per core (group)
DG = NHG * HD              # 512, channel span per head group
F = 4 * C                  # 4096 FFN hidden
FG = F // 2                # 2048 per core
P = 128                    # partitions
NEG = -1e30
EPS = 1e-5
SCALE = HD ** -0.5         # 0.125

NT = T // P                # 8 tiles along T (as partitions) or C
TCH = 512                  # t-chunk (matmul moving free dim)
NTC = T // TCH             # 2 t-chunks
NF = FG // P               # 16 hidden tiles per core


def _ln_tiles(nc, tc, ctx, x_dram, gamma_dram, beta_dram, pool, tag):
    """LayerNorm over the free (T) axis of [C,T]-layout bf16 input; returns 8
    resident bf16 tiles [128, T]. Stats are batched into [128, NT] ops.
    gamma/beta dram are [128, NT] (column ci = channel slice ci)."""
    xpool = ctx.enter_context(tc.tile_pool(name=f"{tag}_x", bufs=NT))
    spool = ctx.enter_context(tc.tile_pool(name=f"{tag}_s", bufs=2))
    vpool = ctx.enter_context(tc.tile_pool(name=f"{tag}_v", bufs=1))

    gam = vpool.tile([P, NT], f32, tag="gam")
    bet = vpool.tile([P, NT], f32, tag="bet")
    nc.sync.dma_start(gam[:], gamma_dram[:])
    nc.sync.dma_start(bet[:], beta_dram[:])
    epst = vpool.tile([P, 1], f32, tag="eps")
    nc.vector.memset(epst[:], EPS)

    sums = vpool.tile([P, NT], f32, tag="sums")
    sumsq = vpool.tile([P, NT], f32, tag="sumsq")
    xts = []
    for ci in range(NT):
        xt = xpool.tile([P, T], bf16, tag="xt")
        nc.sync.dma_start(xt[:], x_dram[P * ci:P * (ci + 1), :])
        xts.append(xt)
        nc.vector.reduce_sum(sums[:, ci:ci + 1], xt[:], axis=mybir.AxisListType.X)
        scratch = spool.tile([P, T], f32, tag="scratch")
        nc.scalar.activation(scratch[:], xt[:], mybir.ActivationFunctionType.Square,
                             accum_out=sumsq[:, ci:ci + 1])
    m = vpool.tile([P, NT], f32, tag="m")
    nc.vector.tensor_scalar_mul(m[:], sums[:], 1.0 / T)
    t1 = vpool.tile([P, NT], f32, tag="t1")
    nc.vector.tensor_mul(t1[:], sums[:], m[:])
    nc.vector.tensor_sub(t1[:], sumsq[:], t1[:])
    nc.vector.tensor_scalar_mul(t1[:], t1[:], 1.0 / (T - 1))  # unbiased var
    std = vpool.tile([P, NT], f32, tag="std")
    nc.scalar.activation(std[:], t1[:], mybir.ActivationFunctionType.Sqrt,
                         bias=epst[:])
    rstd = vpool.tile([P, NT], f32, tag="rstd")
    nc.vector.reciprocal(rstd[:], std[:])
    a = vpool.tile([P, NT], f32, tag="a")
    nc.vector.tensor_mul(a[:], rstd[:], gam[:])
    b0 = vpool.tile([P, NT], f32, tag="b0")
    nc.vector.tensor_mul(b0[:], m[:], a[:])
    nc.vector.tensor_sub(b0[:], bet[:], b0[:])

    h_tiles = []
    for ci in range(NT):
        ht = pool.tile([P, T], bf16, tag=f"{tag}_h{ci}")
        nc.scalar.activation(ht[:], xts[ci][:], mybir.ActivationFunctionType.Identity,
                             bias=b0[:, ci:ci + 1], scale=a[:, ci:ci + 1])
        h_tiles.append(ht)
    return h_tiles


def build_phase_a():
    nc = bacc.Bacc("TRN2", target_bir_lowering=False, debug=False)
    xT = nc.dram_tensor("xT", [C, T], bf16, kind="ExternalInput")
    wk = nc.dram_tensor("wk", [C, DG], bf16, kind="ExternalInput")
    wv = nc.dram_tensor("wv", [C, DG], bf16, kind="ExternalInput")
    wpT = nc.dram_tensor("wpT", [DG, C], bf16, kind="ExternalInput")
    g1 = nc.dram_tensor("g1", [P, NT], f32, kind="ExternalInput")
    beta1 = nc.dram_tensor("beta1", [P, NT], f32, kind="ExternalInput")
    mask = nc.dram_tensor("mask", [4 * P, TCH], bf16, kind="ExternalInput")
    projT = nc.dram_tensor("projT", [C, T], f32, kind="ExternalOutput")

    with tile.TileContext(nc) as tc, ExitStack() as ctx:
        persist = ctx.enter_context(tc.tile_pool(name="persist", bufs=1))
        psum = ctx.enter_context(tc.tile_pool(name="psum", bufs=1, space="PSUM"))

        # --- LN1 (x DMA queued first) ---
        hT = _ln_tiles(nc, tc, ctx, xT, g1, beta1, persist, "ln1")

        # weight tiles
        wk_sb, wv_sb, wpT_sb = [], [], []
        for ci in range(NT):
            t = persist.tile([P, DG], bf16, tag=f"wk{ci}")
            nc.sync.dma_start(t[:], wk[P * ci:P * (ci + 1), :])
            wk_sb.append(t)
        mask_sb = []
        for mv in range(4):
            t = persist.tile([P, TCH], bf16, tag=f"mask{mv}")
            nc.sync.dma_start(t[:], mask[P * mv:P * (mv + 1), :])
            mask_sb.append(t)
        for ci in range(NT):
            t = persist.tile([P, DG], bf16, tag=f"wv{ci}")
            nc.sync.dma_start(t[:], wv[P * ci:P * (ci + 1), :])
            wv_sb.append(t)
        for pr in range(4):
            t = persist.tile([P, C], bf16, tag=f"wpT{pr}")
            nc.sync.dma_start(t[:], wpT[P * pr:P * (pr + 1), :])
            wpT_sb.append(t)

        # --- qk^T projection: psum [128 (2 heads d), 512 t] ---
        qkT = []
        for pr in range(4):
            t = persist.tile([P, T], bf16, tag=f"qkT{pr}")
            qkT.append(t)
        for pr in range(4):
            for tj in range(NTC):
                ps = psum.tile([P, TCH], f32, tag="big", bufs=6)
                for ci in range(NT):
                    nc.tensor.matmul(ps[:], wk_sb[ci][:, P * pr:P * (pr + 1)],
                                     hT[ci][:, TCH * tj:TCH * (tj + 1)],
                                     start=(ci == 0), stop=(ci == NT - 1))
                nc.vector.tensor_copy(qkT[pr][:, TCH * tj:TCH * (tj + 1)], ps[:])

        # --- v projection into v_aug [128 s, 8*65] (65th col of each head = 1.0) ---
        v_aug = []
        for si in range(NT):
            t = persist.tile([P, NHG * (HD + 1)], bf16, tag=f"vaug{si}")
            v_aug.append(t)
        for si in range(NT):
            nc.vector.memset(v_aug[si][:], 1.0)
            ps = psum.tile([P, DG], f32, tag="big", bufs=6)
            for ci in range(NT):
                nc.tensor.matmul(ps[:], hT[ci][:, P * si:P * (si + 1)], wv_sb[ci][:],
                                 start=(ci == 0), stop=(ci == NT - 1))
            va = v_aug[si].rearrange("p (h c) -> p h c", c=HD + 1)
            nc.vector.tensor_copy(va[:, :, 0:HD],
                                  ps[:].rearrange("p (h c) -> p h c", c=HD))

        # --- attention per head ---
        ppool = ctx.enter_context(tc.tile_pool(name="ppool", bufs=2))
        rpool = ctx.enter_context(tc.tile_pool(name="rpool", bufs=3))
        den = persist.tile([P, T], f32, tag="den")   # row h = denom of head h

        attnT = []
        for pr in range(4):
            t = persist.tile([P, T], bf16, tag=f"attnT{pr}")
            attnT.append(t)

        for h in range(NHG):
            pr, off = h // 2, 64 * (h % 2)
            # scores + exp -> pT tiles [128 s, T t] bf16
            pT = [ppool.tile([P, T], bf16, tag=f"pT{si}", name=f"pT{si}")
                  for si in range(NT)]
            for tj in range(NTC):
                for si in range(4 * tj + 4):
                    ps = psum.tile([P, TCH], f32, tag="big", bufs=6)
                    nc.tensor.matmul(
                        ps[:],
                        qkT[pr][off:off + 64, P * si:P * (si + 1)],
                        qkT[pr][off:off + 64, TCH * tj:TCH * (tj + 1)],
                        start=True, stop=True, tile_position=(off, 0))
                    m = si - 4 * tj
                    tbase = TCH * tj
                    nc.scalar.activation(pT[si][:, tbase:tbase + TCH], ps[:],
                                         mybir.ActivationFunctionType.Exp,
                                         scale=SCALE)
                    if m >= 0:
                        # multiplicative causal mask (zeros invalid + diagonal)
                        w = P * (m + 1)
                        sl = pT[si][:, tbase:tbase + w]
                        nc.vector.tensor_mul(sl, sl, mask_sb[m][:, 0:w])
            # AV: psum [65, 512] = [attnU rows 0..63, denom row 64]
            for tj in range(NTC):
                ps = psum.tile([65, TCH], f32, tag="av", bufs=2)
                nsi = 4 * tj + 4
                for si in range(nsi):
                    nc.tensor.matmul(ps[:],
                                     v_aug[si][:, (HD + 1) * h:(HD + 1) * (h + 1)],
                                     pT[si][:, TCH * tj:TCH * (tj + 1)],
                                     start=(si == 0), stop=(si == nsi - 1))
                # unnormalized attn^T -> SBUF (normalized in place later)
                nc.vector.tensor_copy(attnT[pr][off:off + 64, TCH * tj:TCH * (tj + 1)],
                                      ps[0:64, :])
                # denom row -> staging (ACT reads psum), then DMA into den row h
                tmp = rpool.tile([P, TCH], f32, tag="tmp")
                nc.scalar.activation(tmp[64:65, :], ps[64:65, :],
                                     mybir.ActivationFunctionType.Copy)
                hr = 32 * (h // 4) + (h % 4)
                nc.sync.dma_start(den[hr:hr + 1, TCH * tj:TCH * (tj + 1)], tmp[64:65, :])

        # two-batch reciprocal + normalize (batch 1 overlaps heads 4-7)
        rden = persist.tile([P, T], f32, tag="rden")
        for hb in range(2):
            h0 = 4 * hb
            r0 = 32 * hb
            nc.vector.reciprocal(rden[r0:r0 + 4, :], den[r0:r0 + 4, :])
            for h in range(h0, h0 + 4):
                pr, off = h // 2, 64 * (h % 2)
                hr = 32 * hb + (h % 4)
                for tj in range(NTC):
                    bst = rpool.tile([1, TCH], f32, tag="bst")
                    nc.sync.dma_start(bst[:], rden[hr:hr + 1, TCH * tj:TCH * (tj + 1)])
                    R = rpool.tile([P, TCH], f32, tag="R")
                    nc.gpsimd.partition_broadcast(R[:], bst[:])
                    sl = attnT[pr][off:off + 64, TCH * tj:TCH * (tj + 1)]
                    nc.vector.tensor_mul(sl, sl, R[off:off + 64, :])

        # --- output projection (partial over this core's 512 channels) ---
        opool = ctx.enter_context(tc.tile_pool(name="opool", bufs=2))
        for c2 in range(NT):
            pss = [psum.tile([P, TCH], f32, tag="big", bufs=6, name=f"ps{tj}")
                   for tj in range(NTC)]
            for pr in range(4):
                for tj in range(NTC):
                    nc.tensor.matmul(pss[tj][:],
                                     wpT_sb[pr][:, P * c2:P * (c2 + 1)],
                                     attnT[pr][:, TCH * tj:TCH * (tj + 1)],
                                     start=(pr == 0), stop=(pr == 3))
            ot = opool.tile([P, T], f32, tag="ot")
            for tj in range(NTC):
                nc.vector.tensor_copy(ot[:, TCH * tj:TCH * (tj + 1)], pss[tj][:])
            nc.sync.dma_start(projT[P * c2:P * (c2 + 1), :], ot[:])

    nc.compile()
    return nc


def build_phase_b():
    nc = bacc.Bacc("TRN2", target_bir_lowering=False, debug=False)
    x2T = nc.dram_tensor("x2T", [C, T], bf16, kind="ExternalInput")
    # fi-major W1^T: row block fi is [128, 1024] with element [p, 128*ci+q] =
    # W1T[128*ci+p, 128*fi+q]
    w1f = nc.dram_tensor("w1f", [FG, C], bf16, kind="ExternalInput")
    b1 = nc.dram_tensor("b1", [P, NF], f32, kind="ExternalInput")
    w2T = nc.dram_tensor("w2T", [FG, C], bf16, kind="ExternalInput")
    g2 = nc.dram_tensor("g2", [P, NT], f32, kind="ExternalInput")
    beta2 = nc.dram_tensor("beta2", [P, NT], f32, kind="ExternalInput")
    ffpT = nc.dram_tensor("ffpT", [C, T], f32, kind="ExternalOutput")

    with tile.TileContext(nc) as tc, ExitStack() as ctx:
        persist = ctx.enter_context(tc.tile_pool(name="persist", bufs=1))
        psum = ctx.enter_context(tc.tile_pool(name="psum", bufs=1, space="PSUM"))

        h2T = _ln_tiles(nc, tc, ctx, x2T, g2, beta2, persist, "ln2")

        # --- FFN1 + ReLU -> reluT [f, t] bf16 (W1 tiles streamed fi-major) ---
        wpool = ctx.enter_context(tc.tile_pool(name="wpool", bufs=4))
        relu = []
        for fi in range(NF):
            t = persist.tile([P, T], bf16, tag=f"relu{fi}")
            relu.append(t)
        b1_sb = persist.tile([P, NF], f32, tag="b1")
        nc.sync.dma_start(b1_sb[:], b1[:])
        for fi in range(NF):
            wt = wpool.tile([P, C], bf16, tag="w1f")
            nc.sync.dma_start(wt[:], w1f[P * fi:P * (fi + 1), :])
            pss = [psum.tile([P, TCH], f32, tag="f1", bufs=4, name=f"ps{tj}")
                   for tj in range(NTC)]
            for ci in range(NT):
                for tj in range(NTC):
                    nc.tensor.matmul(pss[tj][:],
                                     wt[:, P * ci:P * (ci + 1)],
                                     h2T[ci][:, TCH * tj:TCH * (tj + 1)],
                                     start=(ci == 0), stop=(ci == NT - 1))
            for tj in range(NTC):
                nc.scalar.activation(relu[fi][:, TCH * tj:TCH * (tj + 1)], pss[tj][:],
                                     mybir.ActivationFunctionType.Relu,
                                     bias=b1_sb[:, fi:fi + 1])

        # --- FFN2 (partial) ---
        w2_sb = []
        for fi in range(NF):
            t = persist.tile([P, C], bf16, tag=f"w2T{fi}")
            nc.sync.dma_start(t[:], w2T[P * fi:P * (fi + 1), :])
            w2_sb.append(t)
        opool = ctx.enter_context(tc.tile_pool(name="opool", bufs=2))
        for c2 in range(NT):
            pss = [psum.tile([P, TCH], f32, tag="f2", bufs=4, name=f"ps{tj}")
                   for tj in range(NTC)]
            for fi in range(NF):
                for tj in range(NTC):
                    nc.tensor.matmul(pss[tj][:],
                                     w2_sb[fi][:, P * c2:P * (c2 + 1)],
                                     relu[fi][:, TCH * tj:TCH * (tj + 1)],
                                     start=(fi == 0), stop=(fi == NF - 1))
            ot = opool.tile([P, T], f32, tag="ot")
            for tj in range(NTC):
                nc.vector.tensor_copy(ot[:, TCH * tj:TCH * (tj + 1)], pss[tj][:])
            nc.sync.dma_start(ffpT[P * c2:P * (c2 + 1), :], ot[:])

    nc.compile()
    return nc


_CACHE = {}
TRACE = [False]
EXEC_NS = []


def _get_kernels():
    if "a" not in _CACHE:
        _CACHE["a"] = build_phase_a()
        _CACHE["b"] = build_phase_b()
    return _CACHE["a"], _CACHE["b"]


def _mask01():
    """4 multiplicative [128, 512] bf16 mask variants, stacked [4*128, 512].
    Variant m: cols < 128*m -> 0; diagonal block [128m, 128m+128): keep
    s <= t (local); later cols -> 1 (never multiplied)."""
    out = np.ones((4, P, TCH), np.float32)
    sl = np.arange(P)[:, None]
    tl = np.arange(P)[None, :]
    tri = (sl <= tl).astype(np.float32)
    for m in range(4):
        out[m, :, :P * m] = 0.0
        out[m, :, P * m:P * (m + 1)] = tri
    return out.reshape(4 * P, TCH)


def _bfc(a):
    return np.ascontiguousarray(a).astype(ml_dtypes.bfloat16)


def _pcol(a):
    """[C] vector -> [128, 8] tile, column ci = slice ci."""
    return np.ascontiguousarray(
        np.asarray(a, np.float32).reshape(NT, P).T, dtype=np.float32)


def _w1f_layout(W1T_g):
    """[C, FG] W1^T slice -> fi-major [FG, C] blocks (see build_phase_b)."""
    out = np.empty((FG, C), np.float32)
    for fi in range(NF):
        blk = W1T_g[:, P * fi:P * (fi + 1)]          # [C, 128]
        out[P * fi:P * (fi + 1)] = (
            blk.reshape(NT, P, P).transpose(1, 0, 2).reshape(P, C))
    return out


def prep_a(ins, core):
    b, g = core // 2, core % 2
    heads = range(NHG * g, NHG * (g + 1))
    Wk = np.asarray(ins["Wk"], np.float32)
    Wv = np.asarray(ins["Wv"], np.float32)
    Wp = np.asarray(ins["Wp"], np.float32)
    x = np.asarray(ins["x"], np.float32)
    return {
        "xT": _bfc(x[b].T),
        "wk": _bfc(np.concatenate([Wk[h] for h in heads], axis=1)),
        "wv": _bfc(np.concatenate([Wv[h] for h in heads], axis=1)),
        "wpT": _bfc(Wp.T[DG * g:DG * (g + 1), :]),
        "g1": _pcol(ins["g1"]),
        "beta1": _pcol(ins["beta1"]),
        "mask": _bfc(_mask01()),
    }


def prep_b(ins, x2, core):
    b, g = core // 2, core % 2
    W1T_g = np.asarray(ins["W1"], np.float32).T[:, FG * g:FG * (g + 1)]
    return {
        "x2T": _bfc(x2[b].T),
        "w1f": _bfc(_w1f_layout(W1T_g)),
        "b1": np.ascontiguousarray(np.asarray(ins["b1"], np.float32)
                                   [FG * g:FG * (g + 1)].reshape(NF, P).T),
        "w2T": _bfc(np.asarray(ins["W2"], np.float32).T[FG * g:FG * (g + 1), :]),
        "g2": _pcol(ins["g2"]),
        "beta2": _pcol(ins["beta2"]),
    }


def kernel(x, Wk, Wv, Wp, bp, W1, b1, W2, b2, g1, beta1, g2, beta2):
    from concourse.bass_utils import run_bass_kernel_spmd

    ins = dict(x=x, Wk=Wk, Wv=Wv, Wp=Wp, bp=bp, W1=W1, b1=b1, W2=W2, b2=b2,
               g1=g1, beta1=beta1, g2=g2, beta2=beta2)
    nc_a, nc_b = _get_kernels()
    cores = list(range(8))
    x = np.asarray(x, dtype=np.float32)

    # ---- Phase A ----
    in_maps_a = [prep_a(ins, c) for c in cores]
    ra = run_bass_kernel_spmd(nc_a, in_maps_a, cores, trace=TRACE[0])
    if TRACE[0]:
        EXEC_NS.append(ra.exec_time_ns)
        print("phase A exec_time_ns:", ra.exec_time_ns)
    res_a = ra.results

    x2 = np.empty_like(x)
    for b in range(B):
        x2[b] = (x[b] + res_a[2 * b]["projT"].T + res_a[2 * b + 1]["projT"].T
                 + np.asarray(bp, np.float32)[None, :])

    # ---- Phase B ----
    in_maps_b = [prep_b(ins, x2, c) for c in cores]
    rb = run_bass_kernel_spmd(nc_b, in_maps_b, cores, trace=TRACE[0])
    if TRACE[0]:
        EXEC_NS.append(rb.exec_time_ns)
        print("phase B exec_time_ns:", rb.exec_time_ns)
    res_b = rb.results

    out = np.empty_like(x)
    for b in range(B):
        out[b] = (x2[b] + res_b[2 * b]["ffpT"].T + res_b[2 * b + 1]["ffpT"].T
                  + np.asarray(b2, np.float32)[None, :])
    return out


# hook for test.py: per-core numpy input prep used by the CoreSim path
def sim_feed_a(sim, ins, core):
    for k, v in prep_a(ins, core).items():
        sim.tensor(k)[:] = v


def sim_feed_b(sim, ins, x2, core):
    for k, v in prep_b(ins, x2, core).items():
        sim.tensor(k)[:] = v
